# revision 39
# baseline (speedup 1.0000x reference)
"""Trainium2 Bass kernel for nn_ExperimentalLayer9 (dense transformer layer).

Layer: x + gelu(attn(x)) @ Wf with
  Q = split_heads(x), K = split_heads(x@Wk+bk), V = split_heads(x@Wv+bv)
  causal softmax (no 1/sqrt(d) scale), exact-erf gelu, residual add.

Sharding over 8 NeuronCores: 2 batch groups x 4-way head/tensor parallel.
Core c handles batch b=c//4 and heads [4r, 4r+4) with r=c%4.

Schedule (v9): everything shares one PSUM budget and the PE FIFO is kept
dense by pulling "filler" work units between score tiles:
 - q tile 0 (rows 0-1023): V-projection tiles are the fillers while the
   scalar engine streams exp.
 - q tile 1a (rows 1024-1535): FF row-groups 0/1 (+ their ReduceScatters)
   are the fillers.
 - q tile 1b (rows 1536-2047): FF row-group 2 + RS are the fillers, so
   only FF g3 + one RS remain after the last attention matmul.
 - attention@V runs flipped (V stationary) so o^T lands pre-transposed;
   softmax denominator via bf16 k-sums + all-ones matmul broadcast +
   reciprocal_approx_fast; bv folded in post-normalize (softmax rows sum
   to 1); residual (x+bf)/4 pre-added to every partial so the bf16 RS
   carries it; outputs hop DRAM->SBUF->DRAM to avoid slow DRAM->DRAM.
"""

import numpy as np
import ml_dtypes

import concourse.bass as bass
import concourse.mybir as mybir
import concourse.tile as tile
from concourse import bacc
from concourse import bass_utils

# Problem shapes (hardcoded per contest contract).
B, S, D, H, DHID = 2, 2048, 1024, 16, 4096
NCORES = 8
GROUP = 4              # cores per batch group
HPC = 4                # heads per core
DK = 64                # q/k head dim
DV = 256               # v head dim
DKS = HPC * DK         # 256  k-slice per core
DVS = HPC * DV         # 1024 v/hidden slice per core
ROWS = S // GROUP      # 512  output rows per core after ReduceScatter
NM = D // 128          # 8    contraction chunks over d_model
NST = S // 128         # 16   s tiles of 128
# FF row groups: (row0, nrows). Each reduces over a 4-rank RS; rank r
# receives rows [row0 + (nrows//4)*r, ...). Last groups are small so the
# final collective is cheap.
GROUPS = [(0, 512), (512, 512), (1024, 512), (1536, 256), (1792, 256)]
GOFF = [0, 128, 256, 384, 448]  # output row offset per group (nrows//4 cumsum)

BF16 = mybir.dt.bfloat16
F32 = mybir.dt.float32
AF = mybir.ActivationFunctionType
RG = [[0, 1, 2, 3], [4, 5, 6, 7]]

bf16 = ml_dtypes.bfloat16

_compiled = None


def build_program():
    nc = bacc.Bacc(
        "TRN2",
        target_bir_lowering=False,
        debug=False,
        enable_asserts=True,
        num_devices=NCORES,
    )

    xT = nc.dram_tensor("xT", [D, S], BF16, kind="ExternalInput").ap()
    xresq = nc.dram_tensor("xresq", [S, D], BF16, kind="ExternalInput").ap()
    wk = nc.dram_tensor("wk", [D, DKS], BF16, kind="ExternalInput").ap()
    wv = nc.dram_tensor("wv", [D, DVS], BF16, kind="ExternalInput").ap()
    wf = nc.dram_tensor("wf", [DVS, D], BF16, kind="ExternalInput").ap()
    bkb = nc.dram_tensor("bkb", [1, DKS], BF16, kind="ExternalInput").ap()
    bvp = nc.dram_tensor("bvp", [128, NM], F32, kind="ExternalInput").ap()
    maskt = nc.dram_tensor("maskt", [128, 128], BF16, kind="ExternalInput").ap()
    out = nc.dram_tensor("out", [ROWS, D], BF16, kind="ExternalOutput").ap()

    with tile.TileContext(nc) as tc:
        _body(nc, tc, xT, xresq, wk, wv, wf, bkb, bvp, maskt, out)

    nc.compile()
    return nc


def _body(nc, tc, xT, xresq, wk, wv, wf, bkb, bvp, maskt, out):
    with (
        tc.tile_pool(name="const", bufs=1) as constp,
        tc.tile_pool(name="kv", bufs=1) as kvp,
        tc.tile_pool(name="gotp", bufs=1) as gotp,
        tc.tile_pool(name="wfp", bufs=1) as wfp,
        tc.tile_pool(name="accp", bufs=2) as accp,
        tc.tile_pool(name="rcpp", bufs=1) as rcpp,
        tc.tile_pool(name="dram", bufs=1, space="DRAM") as dramp,
        tc.tile_pool(name="psS", bufs=2, space="PSUM") as psS,
        tc.tile_pool(name="psV", bufs=1, space="PSUM") as psV,
        tc.tile_pool(name="psF", bufs=2, space="PSUM") as psF,
    ):
        # ---- constants ------------------------------------------------
        ones_sb = constp.tile([1, 512], BF16)
        nc.vector.memset(ones_sb[:], 1.0)
        allones = constp.tile([128, 128], BF16)
        nc.vector.memset(allones[:], 1.0)
        mask_sb = constp.tile([128, 128], BF16)
        nc.scalar.dma_start(mask_sb[:], maskt[:])
        bk_sb = constp.tile([1, DKS], BF16)
        nc.scalar.dma_start(bk_sb[:], bkb[:])
        bv_sb = constp.tile([128, NM], F32)
        nc.scalar.dma_start(bv_sb[:], bvp[:])

        warm_in = dramp.tile([4, 16], BF16, tag="warm_in")
        warm_out = dramp.tile([1, 16], BF16, tag="warm_out")
        nc.gpsimd.dma_start(
            warm_in[:].rearrange("a b -> (a b)")[None, :], ones_sb[0:1, 0:64]
        )
        nc.gpsimd.collective_compute(
            "ReduceScatter",
            mybir.AluOpType.add,
            replica_groups=RG,
            ins=[warm_in.opt()],
            outs=[warm_out.opt()],
        )

        # ---- persistent SBUF + loads ----------------------------------
        # q rows are xT rows 0..255 (host rolls the d axis per core)
        qT_sb = kvp.tile([128, 2 * S], BF16)
        kt_sb = kvp.tile([128, 2 * S], BF16)   # K^T rows dk%128, chunk dk//128
        # V: col = kt*1024 + h*256 + dv   (per 128-row k tile)
        v_sb = kvp.tile([128, NST * DVS], BF16)
        got_sb = gotp.tile([128, NM * S], BF16)  # gelu(o)^T, hc-major x q
        wf_sb = wfp.tile([128, NM * D], BF16)
        for m in range(NM):  # gpsimd queue after the warmup collective
            nc.gpsimd.dma_start(
                wf_sb[:, m * D : (m + 1) * D], wf[m * 128 : (m + 1) * 128, :]
            )

        partials = [
            dramp.tile([n, D], BF16, tag=f"part{g}", name=f"part{g}")
            for g, (r0, n) in enumerate(GROUPS)
        ]
        rss = [
            dramp.tile([n // 4, D], BF16, tag=f"rs{g}", name=f"rs{g}")
            for g, (r0, n) in enumerate(GROUPS)
        ]

        # ---- emission helpers -----------------------------------------
        def emit_scores_pair(pair, qb, qw, exps2, acc2, filler=None, rate=1):
            """Causal scores^T -> exp (no max-sub) -> mask -> bf16 running
            k-sums for q rows [qb, qb+qw).  Two heads run row-tiled on the
            64-row PE halves; `rate` filler units are pulled per (kt, hl)
            to keep the PE fed while the scalar engine streams exp."""
            nkt = (qb + qw) // 128
            co = pair * S
            for kt in range(nkt):
                trel = kt * 128 - qb
                toff = max(trel, 0)
                for hl in range(2):
                    po = 64 * hl
                    for qh in range(max(1, qw // 512)):
                        a = max(qh * 512, toff)
                        b_ = min(qh * 512 + 512, qw)
                        if a >= b_:
                            continue
                        ps = psS.tile([128, 512], F32, tag="st", name="st")
                        nc.tensor.matmul(
                            ps[:, a - qh * 512 : b_ - qh * 512],
                            kt_sb[po : po + 64, co + kt * 128 : co + kt * 128 + 128],
                            qT_sb[po : po + 64, co + qb + a : co + qb + b_],
                            start=True,
                            stop=True,
                            tile_position=(po, 0),
                        )
                        nc.scalar.activation(
                            exps2[hl][:, kt * qw + a : kt * qw + b_],
                            ps[:, a - qh * 512 : b_ - qh * 512],
                            AF.Exp,
                        )
                    if trel >= 0:  # mask the diagonal 128x128 block
                        blk = exps2[hl][:, kt * qw + toff : kt * qw + toff + 128]
                        nc.vector.tensor_mul(blk, blk, mask_sb[:])
                    # bf16 running sum over k tiles (softmax denominator)
                    if kt == 0:
                        nc.vector.tensor_copy(
                            acc2[hl][:, 0:qw], exps2[hl][:, 0:qw]
                        )
                    else:
                        nc.vector.tensor_add(
                            acc2[hl][:, toff:qw],
                            acc2[hl][:, toff:qw],
                            exps2[hl][:, kt * qw + toff : (kt + 1) * qw],
                        )
                    if filler is not None:
                        for _ in range(rate):
                            next(filler, None)

        def emit_head(h, qb, qw, exps, acc, filler=None):
            """Flipped AV (V stationary) + l broadcast + normalize for one
            head over q rows [qb, qb+qw)."""
            nkt = (qb + qw) // 128
            psvs = []
            rcp = rcpp.tile([128, 1024], F32, tag=f"r{h % 2}", name="rcp")
            for c in range(2):
                psv = psV.tile([128, qw], F32, tag=f"av{c}", name="psv")
                for kt in range(nkt):
                    toff = max(kt * 128 - qb, 0)
                    vcol = kt * DVS + h * 256 + c * 128
                    for qh in range(max(1, qw // 512)):
                        a = max(qh * 512, toff)
                        b_ = min(qh * 512 + 512, qw)
                        if a >= b_:
                            continue
                        nc.tensor.matmul(
                            psv[:, a:b_],
                            v_sb[:, vcol : vcol + 128],
                            exps[:, kt * qw + a : kt * qw + b_],
                            start=(kt == 0),
                            stop=(kt == nkt - 1),
                            skip_group_check=True,
                        )
                    if c == 0 and filler is not None:
                        next(filler, None)
                psvs.append(psv)
                if c == 0:
                    # l(q) replicated across partitions: allones.T @ acc;
                    # emitted here so the fast reciprocal overlaps chunk 1.
                    for half in range(max(1, qw // 512)):
                        w = min(512, qw - half * 512)
                        rb = psS.tile([128, 512], F32, tag="st", name="rb")
                        nc.tensor.matmul(
                            rb[:, 0:w], allones[:],
                            acc[:, half * 512 : half * 512 + w],
                            start=True, stop=True,
                        )
                        nc.vector.reciprocal_approx_fast(
                            rcp[:, half * 512 : half * 512 + w], rb[:, 0:w]
                        )
            for c in range(2):
                hc = 2 * h + c
                gs = got_sb[:, hc * S + qb : hc * S + qb + qw]
                nc.vector.tensor_mul(gs, psvs[c][:], rcp[:, 0:qw])
                nc.vector.tensor_scalar_add(gs, gs, bv_sb[:, hc : hc + 1])

        def emit_gelu(qb, qw):
            for hc in range(NM):
                gs = got_sb[:, hc * S + qb : hc * S + qb + qw]
                nc.scalar.activation(gs, gs, AF.Gelu)

        def attn_tile(qb, qw, epool, filler=None, rate=1, head_fill=True,
                      sc_fillers=None):
            """One q tile of attention (both head pairs).  `filler` is
            pulled inside score loops (and AV chunk-0 loops when
            head_fill — only safe for psF-based FF units).  sc_fillers
            optionally gives a dedicated filler per score pair."""
            nkt = (qb + qw) // 128
            for pair in range(2):
                e2 = [
                    epool.tile([128, nkt * qw], BF16, tag=f"e{hl}", name="e2")
                    for hl in range(2)
                ]
                a2 = [
                    accp.tile([128, 1024], BF16, tag=f"a{hl}", name="a2")
                    for hl in range(2)
                ]
                scf = sc_fillers[pair] if sc_fillers else filler
                emit_scores_pair(pair, qb, qw, e2, a2, filler=scf, rate=rate)
                if sc_fillers:
                    for _ in scf:  # drain: these units must precede the AVs
                        pass
                hf = filler if head_fill else None
                emit_head(2 * pair, qb, qw, e2[0], a2[0], filler=hf)
                emit_head(2 * pair + 1, qb, qw, e2[1], a2[1], filler=hf)
            if filler is not None:
                for _ in filler:
                    pass
            emit_gelu(qb, qw)

        def ff_gen(g, xrp, fop, xrs=None, store_q=None):
            """FF partial for row group g; one yield per 2 matmuls."""
            store_q = store_q or nc.sync
            row0, nrows = GROUPS[g]
            for cc in range(nrows // 128):
                q0 = row0 + cc * 128
                if xrs is not None:
                    xr = xrs[cc]
                else:
                    xr = xrp.tile([128, D], BF16, tag="xr", name="xr")
                    nc.sync.dma_start(xr[:], xresq[q0 : q0 + 128, :])
                ps0 = psF.tile([128, 512], F32, tag="ff", name="ps0")
                ps1 = psF.tile([128, 512], F32, tag="ff", name="ps1")
                for hc in range(NM):
                    lhsT = got_sb[:, hc * S + q0 : hc * S + q0 + 128]
                    nc.tensor.matmul(
                        ps0[:], lhsT, wf_sb[:, hc * D : hc * D + 512],
                        start=(hc == 0), stop=(hc == NM - 1),
                    )
                    nc.tensor.matmul(
                        ps1[:], lhsT, wf_sb[:, hc * D + 512 : hc * D + 1024],
                        start=(hc == 0), stop=(hc == NM - 1),
                    )
                    yield
                # pre-add (x+bf)/4 so the RS carries the residual
                fo = fop.tile([128, D], BF16, tag="fo", name="fo")
                nc.vector.tensor_add(fo[:, 0:512], ps0[:], xr[:, 0:512])
                nc.vector.tensor_add(fo[:, 512:1024], ps1[:], xr[:, 512:1024])
                store_q.dma_start(partials[g][cc * 128 : (cc + 1) * 128, :], fo[:])

        def emit_rs(g, fop):
            nc.gpsimd.collective_compute(
                "ReduceScatter",
                mybir.AluOpType.add,
                replica_groups=RG,
                ins=[partials[g].opt()],
                outs=[rss[g].opt()],
            )
            # RS-ordered fast DRAM->SBUF hop on gpsimd, then SBUF->DRAM on
            # the sync queue (direct DRAM->DRAM measures ~13us and would
            # serialize the remaining collectives).
            sh = GROUPS[g][1] // 4
            ot = fop.tile([128, D], BF16, tag="ot", name="ot")
            nc.gpsimd.dma_start(ot[0:sh, :], rss[g][:])
            nc.sync.dma_start(out[GOFF[g] : GOFF[g] + sh, :], ot[0:sh, :])

        # ================ phase A: projections + q tile 0 ==============
        with (
            tc.tile_pool(name="xtp", bufs=1) as xtp,
            tc.tile_pool(name="expj0", bufs=1) as expj0,
        ):
            wk_sb = xtp.tile([128, NM * DKS], BF16)
            for m in range(NM):  # head of the scalar queue: needed first
                nc.scalar.dma_start(
                    wk_sb[:, m * DKS : (m + 1) * DKS], wk[m * 128 : (m + 1) * 128, :]
                )
            xT_sb = xtp.tile([128, NM * S], BF16)
            for m in range(NM):  # split xT across three DMA channels
                q = nc.gpsimd if m == 0 else (nc.sync if m < 4 else nc.scalar)
                q.dma_start(
                    xT_sb[:, m * S : (m + 1) * S], xT[m * 128 : (m + 1) * 128, :]
                )
            for m in range(2):
                nc.sync.dma_start(
                    qT_sb[:, m * S : (m + 1) * S], xT[m * 128 : (m + 1) * 128, :]
                )
            wv_sb = xtp.tile([128, NM * DVS], BF16)
            for m in range(NM):
                nc.scalar.dma_start(
                    wv_sb[:, m * DVS : (m + 1) * DVS], wv[m * 128 : (m + 1) * 128, :]
                )

            # K^T proj: 2 passes of 4 st-tiles, m-outer so the PE starts
            # as soon as xT chunk 0 lands (bias seeds run immediately).
            for dkt in range(2):
                ktile = []
                for st in range(4):
                    pool, tg = (psS, "st") if st < 2 else (psF, "ff")
                    kps = pool.tile([128, 512], F32, tag=tg, name="kps")
                    nc.tensor.matmul(
                        kps[:],
                        bk_sb[:, dkt * 128 : (dkt + 1) * 128],
                        ones_sb[:, 0:512],
                        start=True,
                        stop=False,
                    )
                    ktile.append(kps)
                for m in range(NM):
                    for st in range(4):
                        nc.tensor.matmul(
                            ktile[st][:],
                            wk_sb[:, m * DKS + dkt * 128 : m * DKS + dkt * 128 + 128],
                            xT_sb[:, m * S + st * 512 : m * S + st * 512 + 512],
                            start=False,
                            stop=(m == NM - 1),
                        )
                for st in range(4):
                    nc.vector.tensor_copy(
                        kt_sb[:, dkt * S + st * 512 : dkt * S + st * 512 + 512],
                        ktile[st][:],
                    )

            def v_gen(st0, st1):
                """V proj units (st): 16 matmuls + one wide DVE copy."""
                for st in range(st0, st1):
                    vps = psV.tile([128, 1024], F32, tag=f"av{st % 2}", name="vps")
                    for dvh in range(2):
                        for m in range(NM):
                            nc.tensor.matmul(
                                vps[:, dvh * 512 : dvh * 512 + 512],
                                xT_sb[:, m * S + st * 128 : m * S + st * 128 + 128],
                                wv_sb[:, m * DVS + dvh * 512 : m * DVS + dvh * 512 + 512],
                                start=(m == 0),
                                stop=(m == NM - 1),
                            )
                    nc.vector.tensor_copy(
                        v_sb[:, st * DVS : (st + 1) * DVS], vps[:]
                    )
                    yield

            # V tiles fill the score sections only (their psV allocations
            # may not interleave with AV psum slots); st 0-7 must land
            # before the first AV, st 8-15 are needed only for q >= 1024.
            attn_tile(
                0, 1024, expj0, rate=1, head_fill=False,
                sc_fillers=[v_gen(0, 8), v_gen(8, 16)],
            )

        # ========== phase B: q tiles 1a/1b + FF + RS ==================
        with (
            tc.tile_pool(name="expj1", bufs=1) as expj1,
            tc.tile_pool(name="xrp", bufs=4) as xrp,
            tc.tile_pool(name="fop", bufs=2) as fop,
        ):
            # Preload residual tiles for the tail row group so no DMA reads
            # compete with the tail ReduceScatter.
            xr_tail = []
            for i in range(2):
                xr2 = xrp.tile([128, D], BF16, tag="xr2", bufs=2, name="xr2")
                nc.sync.dma_start(
                    xr2[:], xresq[1792 + i * 128 : 1792 + (i + 1) * 128, :]
                )
                xr_tail.append(xr2)

            def chain_a():
                yield from ff_gen(0, xrp, fop)
                emit_rs(0, fop)
                yield from ff_gen(1, xrp, fop)
                emit_rs(1, fop)

            def chain_b():
                yield from ff_gen(2, xrp, fop)
                emit_rs(2, fop)

            def chain_c():
                yield from ff_gen(3, xrp, fop)
                emit_rs(3, fop)

            attn_tile(1024, 512, expj1, filler=chain_a(), rate=1)
            attn_tile(1536, 256, expj1, filler=chain_b(), rate=1)
            attn_tile(1792, 256, expj1, filler=chain_c(), rate=1)

            # ---- FF tail: the final 256-row group ----
            for _ in ff_gen(4, xrp, fop, xrs=xr_tail, store_q=nc.scalar):
                pass
            emit_rs(4, fop)


def make_in_maps(x, Wk, bk, Wv, bv, Wf, bf):
    """Host-side sharding: returns the per-core input dict list."""
    x = np.asarray(x, np.float32)
    Wk = np.asarray(Wk, np.float32)
    Wv = np.asarray(Wv, np.float32)
    Wf = np.asarray(Wf, np.float32)
    bk = np.asarray(bk, np.float32)
    bv = np.asarray(bv, np.float32)
    bf = np.asarray(bf, np.float32)
    mask = np.tril(np.ones((128, 128), np.float32)).T  # mask[k,q]=1 iff k<=q
    in_maps = []
    for c in range(NCORES):
        b, r = c // GROUP, c % GROUP
        xb = x[b]                                    # [S, D]
        # Roll the d axis so this core's q-head rows sit at xT rows 0..255
        # (jointly rolling xT / Wk / Wv rows leaves the contraction
        # invariant and keeps the device program SPMD-identical).
        sh = -DKS * r
        xTr = np.roll(xb.T, sh, axis=0)
        bv_s = bv[DVS * r : DVS * (r + 1)]
        in_maps.append({
            "xT": np.ascontiguousarray(xTr).astype(bf16),
            "xresq": np.ascontiguousarray((xb + bf[None, :]) * 0.25).astype(bf16),
            "wk": np.ascontiguousarray(
                np.roll(Wk[:, DKS * r : DKS * (r + 1)], sh, axis=0)
            ).astype(bf16),
            "wv": np.ascontiguousarray(
                np.roll(Wv[:, DVS * r : DVS * (r + 1)], sh, axis=0)
            ).astype(bf16),
            "wf": np.ascontiguousarray(Wf[DVS * r : DVS * (r + 1), :]).astype(bf16),
            "bkb": bk[None, DKS * r : DKS * (r + 1)].astype(bf16),
            "bvp": np.ascontiguousarray(bv_s.reshape(NM, 128).T).astype(np.float32),
            "maskt": mask.astype(bf16),
        })
    return in_maps


def assemble(results):
    """[8 x [512,1024] bf16] core outputs -> [2,2048,1024] f32."""
    out = np.empty((B, S, D), np.float32)
    for c in range(NCORES):
        b, r = c // GROUP, c % GROUP
        res = np.asarray(results[c]["out"], dtype=np.float32)
        for g, (row0, n) in enumerate(GROUPS):
            sh = n // 4
            out[b, row0 + sh * r : row0 + sh * (r + 1), :] = res[
                GOFF[g] : GOFF[g] + sh
            ]
    return out


def kernel(x, Wk, bk, Wv, bv, Wf, bf, _trace=False, _trace_cores=None):
    global _compiled
    if _compiled is None:
        _compiled = build_program()
    nc = _compiled
    in_maps = make_in_maps(x, Wk, bk, Wv, bv, Wf, bf)
    res = bass_utils.run_bass_kernel_spmd(
        nc,
        in_maps,
        core_ids=list(range(NCORES)),
        trace=_trace,
        trace_cores=_trace_cores,
    )
    out = assemble(res.results)
    kernel.last_result = res
    return out
